# revision 7
# baseline (speedup 1.0000x reference)
"""Trainium2 Bass kernel for nn_DeConv_40029095199190.

Computation: 1x1 conv (256->128) -> 2x nearest upsample -> involution
(5x5, 8 groups x 16 ch, per-pixel kernels generated by conv-bn-relu-conv)
at 128x128.

Structure exploited:
  * conv1x1 commutes with nearest upsample -> the whole kernel-generation
    branch runs at 64x64.
  * A 5x5 involution on a 2x-nearest-upsampled map reads only a 3x3
    neighborhood of distinct small-grid cells; the 25 taps fold into 9
    parity-dependent taps (parity = (y%2, x%2)). The fold is a fixed linear
    map on the span-conv weights -> host-precomputed folded span weights per
    (parity, tap), pre-expanded over the 16 group channels (expansion is
    free on the PE: replicated lhsT columns).
  * Sharding: 8 cores = 4 batches x 2 H-halves; 1-row halo shipped from the
    host in zero-padded x slabs (no inter-core comms). A mask row makes the
    conv bias exact at image-boundary padding.

All inputs ride in ONE blob DMA per core: a PE matmul may carry only a
single sync-wait, so every PE-read tensor must arrive on one DMA lane.
"""

import numpy as np

K = 5
GC = 16
RED = 4
EPS = 1e-5
B, CIN, HS, WS = 4, 256, 64, 64
COUT = 128
G = COUT // GC        # 8
CRED = COUT // RED    # 32
NCORES = 8
ROWS = 34             # slab rows (32 own + 2 halo)
PX = ROWS * WS        # 2176
OWN = 32 * WS         # 2048 own-cell pixels

# blob layout (f32 columns)
O_W1 = 0              # [128, 2*128]
O_WR = 256            # [128, 32]
O_B2E = 288           # [128, 36]
O_X = 324             # [128, 2*PX]
O_MSK = O_X + 2 * PX  # [1, PX] on partition 0
O_B1 = O_MSK + PX     # [1, 128] on partition 0
O_BR = O_B1 + COUT    # [32, 1]
O_W2E = O_BR + 1      # [32, 36*128]
FBLOB = O_W2E + 36 * COUT

_cache = {}


def _build_module():
    import concourse.bass as bass
    import concourse.tile as tile
    import concourse.mybir as mybir

    f32 = mybir.dt.float32
    bf16 = mybir.dt.bfloat16
    AF = mybir.ActivationFunctionType
    OP = mybir.AluOpType

    nc = bass.Bass("TRN2", target_bir_lowering=False, debug=False)
    # Prototype NoOp used by the multi-wait splitting post-pass below.
    _nop_proto = nc.vector.nop().ins

    blob_d = nc.dram_tensor("blob", [128, FBLOB], f32, kind="ExternalInput").ap()
    out_d = nc.dram_tensor("out", [COUT, 64, 128], f32, kind="ExternalOutput").ap()

    with tile.TileContext(nc) as tc:
        from contextlib import ExitStack
        with ExitStack() as ctx:
            big = ctx.enter_context(tc.tile_pool(name="big", bufs=1))
            wxp = ctx.enter_context(tc.tile_pool(name="wxp", bufs=5))
            tmpp = ctx.enter_context(tc.tile_pool(name="tmpp", bufs=3))
            accp = ctx.enter_context(tc.tile_pool(name="accp", bufs=1))
            rowp = ctx.enter_context(tc.tile_pool(name="rowp", bufs=2))
            ph_pool = ctx.enter_context(tc.tile_pool(name="ph", bufs=1, space="PSUM"))
            pr_pool = ctx.enter_context(tc.tile_pool(name="pr", bufs=1, space="PSUM"))
            pw_pool = ctx.enter_context(tc.tile_pool(name="pw", bufs=3, space="PSUM"))

            blob = big.tile([128, FBLOB], f32)
            nc.sync.dma_start(out=blob, in_=blob_d)

            w1v = blob[:, O_W1:O_W1 + 256].rearrange("p (k m) -> p k m", k=2)
            wrv = blob[:, O_WR:O_WR + CRED]
            b2ev = blob[:, O_B2E:O_B2E + 36]
            xv = blob[:, O_X:O_X + 2 * PX].rearrange("p (k f) -> p k f", k=2)
            mskv = blob[0:1, O_MSK:O_MSK + PX]
            b1v = blob[0:1, O_B1:O_B1 + COUT]
            brv = blob[0:CRED, O_BR:O_BR + 1]
            w2ev = blob[0:CRED, O_W2E:O_W2E + 36 * COUT]

            # ---- h tiles ----
            h32 = big.tile([128, PX], f32)                  # [c, row*64+v]
            hbfA = big.tile([128, ROWS, 66], bf16)          # data cols 1..64
            hbfB = big.tile([128, ROWS, 64], bf16)          # plain layout
            nc.gpsimd.memset(hbfA[:, :, 0:1], 0.0)
            nc.gpsimd.memset(hbfA[:, :, 65:66], 0.0)

            # ---- conv1: h = w1.T @ x + b1 * msk ----
            row_tiles = [(0, 8), (8, 8), (16, 8), (24, 8), (32, 2)]
            for (r0, nr) in row_tiles:
                n = nr * WS
                o = r0 * WS
                ph_t = ph_pool.tile([128, 512], f32, tag="ph")
                pt = ph_t[:, :n]
                nc.tensor.matmul(pt, w1v[:, 0, :], xv[:, 0, o:o + n],
                                 start=True, stop=False)
                nc.tensor.matmul(pt, w1v[:, 1, :], xv[:, 1, o:o + n],
                                 start=False, stop=False)
                nc.tensor.matmul(pt, b1v, mskv[:, o:o + n],
                                 start=False, stop=True)
                nc.scalar.activation(h32[:, o:o + n], pt, AF.Copy)
                nc.scalar.activation(hbfA[:, r0:r0 + nr, 1:65], pt, AF.Copy)
                nc.scalar.activation(hbfB[:, r0:r0 + nr, :], pt, AF.Copy)

            # ---- r = relu(wr.T @ h_own + br) ----
            r_sb = big.tile([CRED, OWN], f32)
            for j in range(4):
                pr_t = pr_pool.tile([CRED, 512], f32, tag="pr")
                nc.tensor.matmul(pr_t, wrv, h32[:, WS + j * 512:WS + (j + 1) * 512],
                                 start=True, stop=True)
                nc.scalar.activation(r_sb[:, j * 512:(j + 1) * 512], pr_t, AF.Relu,
                                     bias=brv)

            # ---- involution: 4 parities x 9 taps ----
            accs = []
            for p in range(4):
                acc = accp.tile([128, OWN], bf16, tag=f"acc{p}")
                accs.append(acc)
                for t in range(9):
                    dy, dx = t // 3 - 1, t % 3 - 1
                    pt_i = p * 9 + t
                    wx = wxp.tile([128, OWN], bf16, tag="wx")
                    for j in range(2):
                        pw_t = pw_pool.tile([128, 1024], f32, tag="pw")
                        for jj in range(2):
                            o = j * 1024 + jj * 512
                            nc.tensor.matmul(
                                pw_t[:, jj * 512:(jj + 1) * 512],
                                w2ev[:, pt_i * COUT:(pt_i + 1) * COUT],
                                r_sb[:, o:o + 512],
                                start=True, stop=True)
                        nc.scalar.activation(wx[:, j * 1024:(j + 1) * 1024], pw_t,
                                             AF.Identity,
                                             bias=b2ev[:, pt_i:pt_i + 1])
                    if dx == 0:
                        hsv = hbfB[:, 1 + dy:33 + dy, :]
                    else:
                        hsv = hbfA[:, 1 + dy:33 + dy, 1 + dx:65 + dx]
                    wx3 = wx.rearrange("p (r v) -> p r v", v=WS)
                    if t == 0:
                        nc.vector.tensor_tensor(
                            acc.rearrange("p (r v) -> p r v", v=WS), wx3, hsv, OP.mult)
                    else:
                        tmp = tmpp.tile([128, OWN], bf16, tag="tmp")
                        nc.vector.tensor_tensor(
                            tmp.rearrange("p (r v) -> p r v", v=WS), wx3, hsv, OP.mult)
                        nc.vector.tensor_tensor(acc, acc, tmp, OP.add)

            # ---- interleave parities + DMA out ----
            for pa in range(2):
                row_t = rowp.tile([128, 32, 128], f32, tag="row")
                for pb in range(2):
                    nc.scalar.activation(
                        row_t.rearrange("p r (v two) -> p r v two", two=2)[:, :, :, pb],
                        accs[pa * 2 + pb].rearrange("p (r v) -> p r v", v=WS),
                        AF.Copy)
                nc.sync.dma_start(out=out_d[:, pa::2, :], in_=row_t)

    _split_multiwaits(nc, _nop_proto)
    return nc


def _split_multiwaits(nc, nop_proto):
    """The walrus build in this container rejects instructions carrying more
    than one sync-wait. Hoist extra waits onto injected same-engine NoOps
    (the sequencer executes them in order, so semantics are unchanged)."""
    import copy as _copy
    import bass_rust

    cnt = 0
    for f in nc.m.functions:
        for blk in f.blocks:
            new_list = []
            changed = False
            for inst in blk.instructions:
                si = inst.sync_info
                if si is not None and len(si.on_wait) > 1:
                    changed = True
                    waits = list(si.on_wait)
                    for w in waits[:-1]:
                        n = _copy.replace(nop_proto, name=f"WSPLIT-{cnt}")
                        cnt += 1
                        n.engine = inst.engine
                        n.sync_info = bass_rust.SyncInfo(on_wait=[w], on_update=[])
                        new_list.append(n)
                    inst.sync_info = bass_rust.SyncInfo(
                        on_wait=[waits[-1]], on_update=list(si.on_update))
                new_list.append(inst)
            if changed:
                blk.instructions = new_list


def _host_prep(inputs):
    x = np.ascontiguousarray(np.asarray(inputs["x"], dtype=np.float32))
    w1x1 = np.asarray(inputs["w1x1"], dtype=np.float32)
    b1x1 = np.asarray(inputs["b1x1"], dtype=np.float32)
    w_red = np.asarray(inputs["w_red"], dtype=np.float32)
    b_red = np.asarray(inputs["b_red"], dtype=np.float32)
    bn_gamma = np.asarray(inputs["bn_gamma"], dtype=np.float32)
    bn_beta = np.asarray(inputs["bn_beta"], dtype=np.float32)
    bn_mean = np.asarray(inputs["bn_mean"], dtype=np.float32)
    bn_var = np.asarray(inputs["bn_var"], dtype=np.float32)
    w_span = np.asarray(inputs["w_span"], dtype=np.float32)
    b_span = np.asarray(inputs["b_span"], dtype=np.float32)

    a = bn_gamma / np.sqrt(bn_var + EPS)
    w_red_f = w_red * a[:, None]
    b_red_f = (b_red - bn_mean) * a + bn_beta

    w_span_g = w_span.reshape(G, K * K, CRED)
    b_span_g = b_span.reshape(G, K * K)
    w2 = np.zeros((2, 2, G, 3, 3, CRED), np.float32)
    b2 = np.zeros((2, 2, G, 3, 3), np.float32)
    for pa in range(2):
        for pb in range(2):
            for ky in range(K):
                dy = (pa + ky - 2) // 2
                for kx in range(K):
                    dx = (pb + kx - 2) // 2
                    w2[pa, pb, :, dy + 1, dx + 1] += w_span_g[:, ky * 5 + kx]
                    b2[pa, pb, :, dy + 1, dx + 1] += b_span_g[:, ky * 5 + kx]

    w2e = np.zeros((CRED, 36 * COUT), np.float32)
    b2e = np.zeros((COUT, 36), np.float32)
    for p in range(4):
        pa, pb = p // 2, p % 2
        for t in range(9):
            ty, tx = t // 3, t % 3
            pt = p * 9 + t
            wexp = np.repeat(w2[pa, pb, :, ty, tx, :], GC, axis=0)  # [COUT, CRED]
            w2e[:, pt * COUT:(pt + 1) * COUT] = wexp.T
            b2e[:, pt] = np.repeat(b2[pa, pb, :, ty, tx], GC)

    shared = np.zeros((128, FBLOB), np.float32)
    # w1: [p, k*128+m] = w1x1.T[k*128+p, m]
    w1T = w1x1.T.reshape(2, 128, COUT)           # [k, p, m]
    shared[:, O_W1:O_W1 + 256] = w1T.transpose(1, 0, 2).reshape(128, 256)
    shared[:, O_WR:O_WR + CRED] = w_red_f.T      # [COUT, CRED]
    shared[:, O_B2E:O_B2E + 36] = b2e
    shared[0, O_B1:O_B1 + COUT] = b1x1
    shared[0:CRED, O_BR] = b_red_f
    shared[0:CRED, O_W2E:O_W2E + 36 * COUT] = w2e

    in_maps = []
    for core in range(NCORES):
        bi, hi = core // 2, core % 2
        u0 = hi * 32
        xs = np.zeros((CIN, ROWS, WS), np.float32)
        msk = np.zeros((ROWS, WS), np.float32)
        lo = max(0, u0 - 1)
        hi_row = min(HS, u0 + 33)
        d0 = lo - (u0 - 1)
        xs[:, d0:d0 + hi_row - lo] = x[bi, :, lo:hi_row]
        msk[d0:d0 + hi_row - lo] = 1.0
        blob = shared.copy()
        blob[:, O_X:O_X + 2 * PX] = \
            xs.reshape(2, 128, PX).transpose(1, 0, 2).reshape(128, 2 * PX)
        blob[0, O_MSK:O_MSK + PX] = msk.reshape(PX)
        in_maps.append({"blob": blob})
    return in_maps


def kernel(**inputs):
    from concourse import bass_utils

    if "nc" not in _cache:
        _cache["nc"] = _build_module()
    nc = _cache["nc"]

    in_maps = _host_prep(inputs)
    res = bass_utils.run_bass_kernel_spmd(nc, in_maps, core_ids=list(range(NCORES)))

    out = np.zeros((B, COUT, 2 * HS, 2 * WS), np.float32)
    for core in range(NCORES):
        bi, hi = core // 2, core % 2
        out[bi, :, hi * 64:(hi + 1) * 64, :] = res.results[core]["out"]
    return out


# revision 13
# speedup vs baseline: 1.4611x; 1.4611x over previous
"""Trainium2 Bass kernel for nn_DeConv_40029095199190.

Computation: 1x1 conv (256->128) -> 2x nearest upsample -> involution
(5x5, 8 groups x 16 ch, per-pixel kernels generated by conv-bn-relu-conv)
at 128x128.

Structure exploited:
  * conv1x1 commutes with nearest upsample -> the whole kernel-generation
    branch runs at 64x64.
  * A 5x5 involution on a 2x-nearest-upsampled map reads only a 3x3
    neighborhood of distinct small-grid cells; the 25 taps fold into 9
    parity-dependent taps (parity = (y%2, x%2)). The fold is a fixed linear
    map on the span-conv weights -> host-precomputed folded span weights per
    (parity, tap), pre-expanded over the 16 group channels (expansion is
    free on the PE: replicated lhsT columns).
  * Sharding: 8 cores = 4 batches x 2 H-halves; 1-row halo shipped from the
    host in zero-padded x slabs (no inter-core comms). A mask row makes the
    conv bias exact at image-boundary padding.

Perf notes (from HW traces):
  * fp32 matmul lowers to 2 HW passes and runs ~8x slower than bf16 ->
    conv1 uses a bf16 hi/lo split (x = x_hi + x_lo, w = w_hi + w_lo,
    keep the three significant cross terms => ~fp32 accuracy at bf16 rate).
  * A PE matmul (or any instruction) in this toolchain may carry only ONE
    sync wait -> all PE-read tensors ride in a single blob DMA, and a
    post-pass splits any remaining multi-wait instruction into NoOps.
  * The ACT engine is the psum->sbuf cast bottleneck -> a subset of taps
    (TB) multiplies straight out of PSUM on the DVE, and the tap add-chains
    are split between DVE and GPSIMD (separate accumulators, merged once).
"""

import numpy as np

K = 5
GC = 16
RED = 4
EPS = 1e-5
B, CIN, HS, WS = 4, 256, 64, 64
COUT = 128
G = COUT // GC        # 8
CRED = COUT // RED    # 32
NCORES = 8
ROWS = 34             # slab rows (32 own + 2 halo)
PX = ROWS * WS        # 2176
OWN = 32 * WS         # 2048 own-cell pixels

# blob layout (f32 columns; bf16 regions pack 2 elems/col)
O_W1HI = 0                      # w1_hi bf16 [128, 2*128]
O_W1LO = O_W1HI + 128           # w1_lo bf16
O_WR = O_W1LO + 128             # wr f32 [128, 32]
O_B2E = O_WR + CRED             # b2e f32 [128, 36]
O_XHI = O_B2E + 36              # x_hi bf16 [128, 2*PX]
O_XLO = O_XHI + PX              # x_lo bf16
O_MSK = O_XLO + PX              # msk bf16 [1, PX]
O_B1HI = O_MSK + PX // 2        # b1_hi bf16 [1, 128]
O_B1LO = O_B1HI + 64            # b1_lo bf16 [1, 128]
O_BR = O_B1LO + 64              # br f32 [32, 1]
O_W2E = O_BR + 1                # w2e bf16 [32, 36*128]
O_B2T = O_W2E + 36 * 64         # b2eT bf16 [1, 36*128] on partition 0
FBLOB = O_B2T + 36 * 64

# taps whose product is taken straight from PSUM (no ACT cast)
TB_TAPS = (0, 4)
# taps whose ADD goes on the GPSIMD side-chain (tap 0 initializes acc)
GP_TAPS = (3, 5, 6, 7)
# route the parity-interleave copies: True -> GPSIMD, False -> ACT
GP_INTERLEAVE = True

_cache = {}


def _build_module():
    import concourse.bass as bass
    import concourse.tile as tile
    import concourse.mybir as mybir

    f32 = mybir.dt.float32
    bf16 = mybir.dt.bfloat16
    AF = mybir.ActivationFunctionType
    OP = mybir.AluOpType

    nc = bass.Bass("TRN2", target_bir_lowering=False, debug=False)
    _nop_proto = nc.vector.nop().ins

    blob_d = nc.dram_tensor("blob", [128, FBLOB], f32, kind="ExternalInput").ap()
    out_d = nc.dram_tensor("out", [COUT, 64, 128], f32, kind="ExternalOutput").ap()

    with tile.TileContext(nc) as tc:
        from contextlib import ExitStack
        with ExitStack() as ctx:
            big = ctx.enter_context(tc.tile_pool(name="big", bufs=1))
            wxp = ctx.enter_context(tc.tile_pool(name="wxp", bufs=5))
            tmpp = ctx.enter_context(tc.tile_pool(name="tmpp", bufs=6))
            accp = ctx.enter_context(tc.tile_pool(name="accp", bufs=1))
            rowp = ctx.enter_context(tc.tile_pool(name="rowp", bufs=2))
            ph_pool = ctx.enter_context(tc.tile_pool(name="ph", bufs=1, space="PSUM"))
            pr_pool = ctx.enter_context(tc.tile_pool(name="pr", bufs=1, space="PSUM"))
            pw_pool = ctx.enter_context(tc.tile_pool(name="pw", bufs=3, space="PSUM"))

            blob = big.tile([128, FBLOB], f32)
            nc.sync.dma_start(out=blob, in_=blob_d)

            def bfv(p0, p1, c0, c1):
                return blob[p0:p1, c0:c1].bitcast(bf16)

            w1hi = bfv(0, 128, O_W1HI, O_W1HI + 128).rearrange("p (k m) -> p k m", k=2)
            w1lo = bfv(0, 128, O_W1LO, O_W1LO + 128).rearrange("p (k m) -> p k m", k=2)
            wrv = blob[:, O_WR:O_WR + CRED]
            b2ev = blob[:, O_B2E:O_B2E + 36]
            xhi = bfv(0, 128, O_XHI, O_XHI + PX).rearrange("p (k f) -> p k f", k=2)
            xlo = bfv(0, 128, O_XLO, O_XLO + PX).rearrange("p (k f) -> p k f", k=2)
            mskv = bfv(0, 1, O_MSK, O_MSK + PX // 2)
            b1hi = bfv(0, 1, O_B1HI, O_B1HI + 64)
            b1lo = bfv(0, 1, O_B1LO, O_B1LO + 64)
            brv = blob[0:CRED, O_BR:O_BR + 1]
            w2ev = bfv(0, CRED, O_W2E, O_W2E + 36 * 64)
            b2tv = bfv(0, 1, O_B2T, O_B2T + 36 * 64)

            # ---- h tiles ----
            h32 = big.tile([128, PX], f32)                  # [c, row*64+v]
            hbfA = big.tile([128, ROWS, 66], bf16)          # data cols 1..64
            hbfB = big.tile([128, ROWS, 64], bf16)          # plain layout
            ones_bf = big.tile([1, OWN], bf16)
            nc.gpsimd.memset(hbfA[:, :, 0:1], 0.0)
            nc.gpsimd.memset(hbfA[:, :, 65:66], 0.0)
            nc.vector.memset(ones_bf, 1.0)

            # ---- conv1: h = (w_hi+w_lo).T @ (x_hi+x_lo) + b1 * msk ----
            row_tiles = [(0, 8), (8, 8), (16, 8), (24, 8), (32, 2)]
            for (r0, nr) in row_tiles:
                n = nr * WS
                o = r0 * WS
                ph_t = ph_pool.tile([128, 512], f32, tag="ph")
                pt = ph_t[:, :n]
                first = True
                for k in range(2):
                    for (wv, xv) in ((w1hi, xhi), (w1hi, xlo), (w1lo, xhi)):
                        nc.tensor.matmul(pt, wv[:, k, :], xv[:, k, o:o + n],
                                         start=first, stop=False)
                        first = False
                nc.tensor.matmul(pt, b1hi, mskv[:, o:o + n], start=False, stop=False)
                nc.tensor.matmul(pt, b1lo, mskv[:, o:o + n], start=False, stop=True)
                nc.scalar.activation(h32[:, o:o + n], pt, AF.Copy)
                nc.scalar.activation(hbfA[:, r0:r0 + nr, 1:65], pt, AF.Copy)
                nc.scalar.activation(hbfB[:, r0:r0 + nr, :], pt, AF.Copy)

            # ---- r = relu(wr.T @ h_own + br) -> bf16 ----
            r_sb = big.tile([CRED, OWN], bf16)
            for j in range(4):
                pr_t = pr_pool.tile([CRED, 512], f32, tag="pr")
                nc.tensor.matmul(pr_t, wrv, h32[:, WS + j * 512:WS + (j + 1) * 512],
                                 start=True, stop=True)
                nc.scalar.activation(r_sb[:, j * 512:(j + 1) * 512], pr_t, AF.Relu,
                                     bias=brv)

            # ---- involution: 4 parities x 9 taps ----
            accs = []
            for p in range(4):
                acc = accp.tile([128, OWN], bf16, tag=f"acc{p}")
                accg = accp.tile([128, OWN], bf16, tag=f"accg{p}")
                accs.append(acc)
                gp_started = False
                for t in range(9):
                    dy, dx = t // 3 - 1, t % 3 - 1
                    pt_i = p * 9 + t
                    if dx == 0:
                        hsv = hbfB[:, 1 + dy:33 + dy, :]
                    else:
                        hsv = hbfA[:, 1 + dy:33 + dy, 1 + dx:65 + dx]

                    if t in TB_TAPS:
                        # multiply straight from PSUM (skip the ACT cast)
                        dsts = []
                        for j in range(2):
                            pw_t = pw_pool.tile([128, 1024], f32, tag="pw")
                            for jj in range(2):
                                o = j * 1024 + jj * 512
                                nc.tensor.matmul(
                                    pw_t[:, jj * 512:(jj + 1) * 512],
                                    w2ev[:, pt_i * COUT:(pt_i + 1) * COUT],
                                    r_sb[:, o:o + 512],
                                    start=True, stop=False)
                                nc.tensor.matmul(
                                    pw_t[:, jj * 512:(jj + 1) * 512],
                                    b2tv[:, pt_i * COUT:(pt_i + 1) * COUT],
                                    ones_bf[:, o:o + 512],
                                    start=False, stop=True)
                            dst = accs[p] if t == 0 else None
                            if dst is None:
                                if not dsts:
                                    tb_tmp = tmpp.tile([128, OWN], bf16, tag="tmp")
                                    dsts = [tb_tmp]
                                dst = dsts[0]
                            else:
                                dsts = [dst]
                            hv = hsv[:, 16 * j:16 * (j + 1), :]
                            dst3 = dst.rearrange("p (r v) -> p r v", v=WS)[
                                :, 16 * j:16 * (j + 1), :]
                            pw3 = pw_t.rearrange("p (r v) -> p r v", v=WS)
                            nc.vector.tensor_tensor(dst3, pw3, hv, OP.mult)
                        prod = dsts[0]
                    else:
                        wx = wxp.tile([128, OWN], bf16, tag="wx")
                        for j in range(2):
                            pw_t = pw_pool.tile([128, 1024], f32, tag="pw")
                            for jj in range(2):
                                o = j * 1024 + jj * 512
                                nc.tensor.matmul(
                                    pw_t[:, jj * 512:(jj + 1) * 512],
                                    w2ev[:, pt_i * COUT:(pt_i + 1) * COUT],
                                    r_sb[:, o:o + 512],
                                    start=True, stop=True)
                            nc.scalar.activation(wx[:, j * 1024:(j + 1) * 1024], pw_t,
                                                 AF.Identity,
                                                 bias=b2ev[:, pt_i:pt_i + 1])
                        wx3 = wx.rearrange("p (r v) -> p r v", v=WS)
                        if t == 0:
                            nc.vector.tensor_tensor(
                                accs[p].rearrange("p (r v) -> p r v", v=WS),
                                wx3, hsv, OP.mult)
                            prod = accs[p]
                        else:
                            prod = tmpp.tile([128, OWN], bf16, tag="tmp")
                            nc.vector.tensor_tensor(
                                prod.rearrange("p (r v) -> p r v", v=WS),
                                wx3, hsv, OP.mult)

                    if t == 0:
                        continue
                    if t in GP_TAPS:
                        if not gp_started:
                            nc.gpsimd.tensor_copy(accg, prod)
                            gp_started = True
                        else:
                            nc.gpsimd.tensor_tensor(accg, accg, prod, OP.add)
                    else:
                        nc.vector.tensor_tensor(acc, acc, prod, OP.add)
                if gp_started:
                    nc.vector.tensor_tensor(acc, acc, accg, OP.add)

            # ---- interleave parities + DMA out ----
            for pa in range(2):
                row_t = rowp.tile([128, 32, 128], f32, tag="row")
                for pb in range(2):
                    dst = row_t.rearrange("p r (v two) -> p r v two", two=2)[:, :, :, pb]
                    src = accs[pa * 2 + pb].rearrange("p (r v) -> p r v", v=WS)
                    if GP_INTERLEAVE:
                        nc.gpsimd.tensor_copy(dst, src)
                    else:
                        nc.scalar.activation(dst, src, AF.Copy)
                nc.sync.dma_start(out=out_d[:, pa::2, :], in_=row_t)

    _split_multiwaits(nc, _nop_proto)
    return nc


def _split_multiwaits(nc, nop_proto):
    """The walrus build in this container rejects instructions carrying more
    than one sync-wait. Hoist extra waits onto injected same-engine NoOps
    (the sequencer executes them in order, so semantics are unchanged)."""
    import copy as _copy
    import bass_rust

    cnt = 0
    for f in nc.m.functions:
        for blk in f.blocks:
            new_list = []
            changed = False
            for inst in blk.instructions:
                si = inst.sync_info
                if si is not None and len(si.on_wait) > 1:
                    changed = True
                    waits = list(si.on_wait)
                    for w in waits[:-1]:
                        n = _copy.replace(nop_proto, name=f"WSPLIT-{cnt}")
                        cnt += 1
                        n.engine = inst.engine
                        n.sync_info = bass_rust.SyncInfo(on_wait=[w], on_update=[])
                        new_list.append(n)
                    inst.sync_info = bass_rust.SyncInfo(
                        on_wait=[waits[-1]], on_update=list(si.on_update))
                new_list.append(inst)
            if changed:
                blk.instructions = new_list


def _pack_bf16(arr):
    """[..., N] f32 -> [..., N/2] f32 whose bytes are the bf16 elements."""
    import ml_dtypes
    bf = np.asarray(arr, dtype=np.float32).astype(ml_dtypes.bfloat16)
    u16 = bf.view(np.uint16).astype(np.uint32)
    lo = u16[..., 0::2]
    hi = u16[..., 1::2]
    return (lo | (hi << 16)).view(np.float32)


def _host_prep(inputs):
    import ml_dtypes
    x = np.ascontiguousarray(np.asarray(inputs["x"], dtype=np.float32))
    w1x1 = np.asarray(inputs["w1x1"], dtype=np.float32)
    b1x1 = np.asarray(inputs["b1x1"], dtype=np.float32)
    w_red = np.asarray(inputs["w_red"], dtype=np.float32)
    b_red = np.asarray(inputs["b_red"], dtype=np.float32)
    bn_gamma = np.asarray(inputs["bn_gamma"], dtype=np.float32)
    bn_beta = np.asarray(inputs["bn_beta"], dtype=np.float32)
    bn_mean = np.asarray(inputs["bn_mean"], dtype=np.float32)
    bn_var = np.asarray(inputs["bn_var"], dtype=np.float32)
    w_span = np.asarray(inputs["w_span"], dtype=np.float32)
    b_span = np.asarray(inputs["b_span"], dtype=np.float32)

    a = bn_gamma / np.sqrt(bn_var + EPS)
    w_red_f = w_red * a[:, None]
    b_red_f = (b_red - bn_mean) * a + bn_beta

    w_span_g = w_span.reshape(G, K * K, CRED)
    b_span_g = b_span.reshape(G, K * K)
    w2 = np.zeros((2, 2, G, 3, 3, CRED), np.float32)
    b2 = np.zeros((2, 2, G, 3, 3), np.float32)
    for pa in range(2):
        for pb in range(2):
            for ky in range(K):
                dy = (pa + ky - 2) // 2
                for kx in range(K):
                    dx = (pb + kx - 2) // 2
                    w2[pa, pb, :, dy + 1, dx + 1] += w_span_g[:, ky * 5 + kx]
                    b2[pa, pb, :, dy + 1, dx + 1] += b_span_g[:, ky * 5 + kx]

    w2e = np.zeros((CRED, 36 * COUT), np.float32)
    b2e = np.zeros((COUT, 36), np.float32)
    for p in range(4):
        pa, pb = p // 2, p % 2
        for t in range(9):
            ty, tx = t // 3, t % 3
            pt = p * 9 + t
            wexp = np.repeat(w2[pa, pb, :, ty, tx, :], GC, axis=0)  # [COUT, CRED]
            w2e[:, pt * COUT:(pt + 1) * COUT] = wexp.T
            b2e[:, pt] = np.repeat(b2[pa, pb, :, ty, tx], GC)

    bf = ml_dtypes.bfloat16
    shared = np.zeros((128, FBLOB), np.float32)
    # conv1 weights, hi/lo split: [p, k*128+m] = w1x1.T[k*128+p, m]
    w1T = w1x1.T.reshape(2, 128, COUT).transpose(1, 0, 2).reshape(128, 256)
    w1T_hi = w1T.astype(bf).astype(np.float32)
    shared[:, O_W1HI:O_W1HI + 128] = _pack_bf16(w1T)
    shared[:, O_W1LO:O_W1LO + 128] = _pack_bf16(w1T - w1T_hi)
    shared[:, O_WR:O_WR + CRED] = w_red_f.T
    shared[:, O_B2E:O_B2E + 36] = b2e
    b1_hi = b1x1.astype(bf).astype(np.float32)
    shared[0, O_B1HI:O_B1HI + 64] = _pack_bf16(b1x1)
    shared[0, O_B1LO:O_B1LO + 64] = _pack_bf16(b1x1 - b1_hi)
    shared[0:CRED, O_BR] = b_red_f
    shared[0:CRED, O_W2E:O_W2E + 36 * 64] = _pack_bf16(w2e)
    shared[0, O_B2T:O_B2T + 36 * 64] = _pack_bf16(np.ascontiguousarray(b2e.T).reshape(36 * COUT))

    in_maps = []
    for core in range(NCORES):
        bi, hi = core // 2, core % 2
        u0 = hi * 32
        xs = np.zeros((CIN, ROWS, WS), np.float32)
        msk = np.zeros((ROWS, WS), np.float32)
        lo_r = max(0, u0 - 1)
        hi_r = min(HS, u0 + 33)
        d0 = lo_r - (u0 - 1)
        xs[:, d0:d0 + hi_r - lo_r] = x[bi, :, lo_r:hi_r]
        msk[d0:d0 + hi_r - lo_r] = 1.0
        blob = shared.copy()
        xsr = xs.reshape(2, 128, PX).transpose(1, 0, 2).reshape(128, 2 * PX)
        xs_hi = xsr.astype(bf).astype(np.float32)
        blob[:, O_XHI:O_XHI + PX] = _pack_bf16(xsr)
        blob[:, O_XLO:O_XLO + PX] = _pack_bf16(xsr - xs_hi)
        blob[0, O_MSK:O_MSK + PX // 2] = _pack_bf16(msk.reshape(PX))
        in_maps.append({"blob": blob})
    return in_maps


def kernel(**inputs):
    from concourse import bass_utils

    if "nc" not in _cache:
        _cache["nc"] = _build_module()
    nc = _cache["nc"]

    in_maps = _host_prep(inputs)
    res = bass_utils.run_bass_kernel_spmd(nc, in_maps, core_ids=list(range(NCORES)))

    out = np.zeros((B, COUT, 2 * HS, 2 * WS), np.float32)
    for core in range(NCORES):
        bi, hi = core // 2, core % 2
        out[bi, :, hi * 64:(hi + 1) * 64, :] = res.results[core]["out"]
    return out


# revision 16
# speedup vs baseline: 2.0969x; 1.4352x over previous
"""Trainium2 Bass kernel for nn_DeConv_40029095199190.

Computation: 1x1 conv (256->128) -> 2x nearest upsample -> involution
(5x5, 8 groups x 16 ch, per-pixel kernels generated by conv-bn-relu-conv)
at 128x128.

Structure exploited:
  * conv1x1 commutes with nearest upsample -> the whole kernel-generation
    branch runs at 64x64.
  * A 5x5 involution on a 2x-nearest-upsampled map reads only a 3x3
    neighborhood of distinct small-grid cells; the 25 taps fold into 9
    parity-dependent taps (parity = (y%2, x%2)). The fold is a fixed linear
    map on the span-conv weights -> host-precomputed folded span weights per
    (parity, tap), pre-expanded over the 16 group channels (expansion is
    free on the PE: replicated lhsT columns).
  * Sharding: 8 cores = 4 batches x 2 H-halves; 1-row halo shipped from the
    host in zero-padded x slabs (no inter-core comms). A mask row makes the
    conv bias exact at image-boundary padding.

Perf notes (from HW traces):
  * fp32 matmul lowers to 2 HW passes and runs ~8x slower than bf16 ->
    conv1 uses a bf16 hi/lo split (x = x_hi + x_lo, w = w_hi + w_lo,
    keep the three significant cross terms => ~fp32 accuracy at bf16 rate).
  * A PE matmul (or any instruction) in this toolchain may carry only ONE
    sync wait -> all PE-read tensors ride in a single blob DMA, and a
    post-pass splits any remaining multi-wait instruction into NoOps.
  * The ACT engine is the psum->sbuf cast bottleneck -> a subset of taps
    (TB) multiplies straight out of PSUM on the DVE, and the tap add-chains
    are split between DVE and GPSIMD (separate accumulators, merged once).
"""

import numpy as np

K = 5
GC = 16
RED = 4
EPS = 1e-5
B, CIN, HS, WS = 4, 256, 64, 64
COUT = 128
G = COUT // GC        # 8
CRED = COUT // RED    # 32
NCORES = 8
ROWS = 34             # slab rows (32 own + 2 halo)
PX = ROWS * WS        # 2176
OWN = 32 * WS         # 2048 own-cell pixels

# blob layout (f32 columns; bf16 regions pack 2 elems/col)
O_W1HI = 0                      # w1_hi bf16 [128, 2*128]
O_W1LO = O_W1HI + 128           # w1_lo bf16
O_WR = O_W1LO + 128             # wr f32 [128, 32]
O_XHI = O_WR + CRED             # x_hi bf16 [128, 2*PX]
O_XLO = O_XHI + PX              # x_lo bf16
O_MSK = O_XLO + PX              # msk bf16 [1, PX]
O_B1HI = O_MSK + PX // 2        # b1_hi bf16 [1, 128]
O_B1LO = O_B1HI + 64            # b1_lo bf16 [1, 128]
O_BR = O_B1LO + 64              # br f32 [32, 1]
O_W2E = O_BR + 1                # w2e33 bf16 [33, 36*128] (row 32 = folded span bias)
FBLOB = O_W2E + 36 * 64

# taps whose product is taken straight from PSUM (no ACT cast)
TB_TAPS = (0, 4)

_cache = {}


def _build_module():
    import concourse.bass as bass
    import concourse.tile as tile
    import concourse.mybir as mybir

    f32 = mybir.dt.float32
    bf16 = mybir.dt.bfloat16
    AF = mybir.ActivationFunctionType
    OP = mybir.AluOpType

    nc = bass.Bass("TRN2", target_bir_lowering=False, debug=False)
    _nop_proto = nc.vector.nop().ins

    blob_d = nc.dram_tensor("blob", [128, FBLOB], f32, kind="ExternalInput").ap()
    out_d = nc.dram_tensor("out", [4, COUT, 32, WS], bf16, kind="ExternalOutput").ap()

    with tile.TileContext(nc) as tc:
        from contextlib import ExitStack
        with ExitStack() as ctx:
            big = ctx.enter_context(tc.tile_pool(name="big", bufs=1))
            wxp = ctx.enter_context(tc.tile_pool(name="wxp", bufs=5))
            tmpp = ctx.enter_context(tc.tile_pool(name="tmpp", bufs=6))
            accp = ctx.enter_context(tc.tile_pool(name="accp", bufs=1))
            rowp = ctx.enter_context(tc.tile_pool(name="rowp", bufs=2))
            ph_pool = ctx.enter_context(tc.tile_pool(name="ph", bufs=1, space="PSUM"))
            pr_pool = ctx.enter_context(tc.tile_pool(name="pr", bufs=1, space="PSUM"))
            pw_pool = ctx.enter_context(tc.tile_pool(name="pw", bufs=3, space="PSUM"))

            blob = big.tile([128, FBLOB], f32)
            nc.sync.dma_start(out=blob, in_=blob_d)

            def bfv(p0, p1, c0, c1):
                return blob[p0:p1, c0:c1].bitcast(bf16)

            w1hi = bfv(0, 128, O_W1HI, O_W1HI + 128).rearrange("p (k m) -> p k m", k=2)
            w1lo = bfv(0, 128, O_W1LO, O_W1LO + 128).rearrange("p (k m) -> p k m", k=2)
            wrv = blob[:, O_WR:O_WR + CRED]
            xhi = bfv(0, 128, O_XHI, O_XHI + PX).rearrange("p (k f) -> p k f", k=2)
            xlo = bfv(0, 128, O_XLO, O_XLO + PX).rearrange("p (k f) -> p k f", k=2)
            mskv = bfv(0, 1, O_MSK, O_MSK + PX // 2)
            b1hi = bfv(0, 1, O_B1HI, O_B1HI + 64)
            b1lo = bfv(0, 1, O_B1LO, O_B1LO + 64)
            brv = blob[0:CRED, O_BR:O_BR + 1]
            w2ev = bfv(0, CRED + 1, O_W2E, O_W2E + 36 * 64)

            # ---- h tiles ----
            h32 = big.tile([128, PX], f32)                  # [c, row*64+v]
            hbfA = big.tile([128, ROWS, 66], bf16)          # data cols 1..64
            hbfB = big.tile([128, ROWS, 64], bf16)          # plain layout
            nc.vector.memset(hbfA[:, :, 0:1], 0.0)
            nc.vector.memset(hbfA[:, :, 65:66], 0.0)

            # ---- conv1: h = (w_hi+w_lo).T @ (x_hi+x_lo) + b1 * msk ----
            row_tiles = [(0, 8), (8, 8), (16, 8), (24, 8), (32, 2)]
            for (r0, nr) in row_tiles:
                n = nr * WS
                o = r0 * WS
                ph_t = ph_pool.tile([128, 512], f32, tag="ph")
                pt = ph_t[:, :n]
                first = True
                for k in range(2):
                    for (wv, xv) in ((w1hi, xhi), (w1hi, xlo), (w1lo, xhi)):
                        nc.tensor.matmul(pt, wv[:, k, :], xv[:, k, o:o + n],
                                         start=first, stop=False)
                        first = False
                nc.tensor.matmul(pt, b1hi, mskv[:, o:o + n], start=False, stop=False)
                nc.tensor.matmul(pt, b1lo, mskv[:, o:o + n], start=False, stop=True)
                nc.scalar.activation(h32[:, o:o + n], pt, AF.Copy)
                nc.scalar.activation(hbfA[:, r0:r0 + nr, 1:65], pt, AF.Copy)
                nc.scalar.activation(hbfB[:, r0:r0 + nr, :], pt, AF.Copy)

            # ---- r = relu(wr.T @ h_own + br) -> bf16 ----
            r_sb = big.tile([CRED + 1, OWN], bf16)
            nc.vector.memset(r_sb[CRED:CRED + 1, :], 1.0)
            for j in range(4):
                pr_t = pr_pool.tile([CRED, 512], f32, tag="pr")
                nc.tensor.matmul(pr_t, wrv, h32[:, WS + j * 512:WS + (j + 1) * 512],
                                 start=True, stop=True)
                nc.scalar.activation(r_sb[0:CRED, j * 512:(j + 1) * 512], pr_t,
                                     AF.Relu, bias=brv)

            # ---- involution: 4 parities x 9 taps ----
            accs = []
            for p in range(4):
                acc = accp.tile([128, OWN], bf16, tag=f"acc{p}")
                accs.append(acc)
                for t in range(9):
                    dy, dx = t // 3 - 1, t % 3 - 1
                    pt_i = p * 9 + t
                    if dx == 0:
                        hsv = hbfB[:, 1 + dy:33 + dy, :]
                    else:
                        hsv = hbfA[:, 1 + dy:33 + dy, 1 + dx:65 + dx]

                    if t in TB_TAPS:
                        # multiply straight from PSUM (skip the ACT cast)
                        dsts = []
                        for j in range(2):
                            pw_t = pw_pool.tile([128, 1024], f32, tag="pw")
                            for jj in range(2):
                                o = j * 1024 + jj * 512
                                nc.tensor.matmul(
                                    pw_t[:, jj * 512:(jj + 1) * 512],
                                    w2ev[:, pt_i * COUT:(pt_i + 1) * COUT],
                                    r_sb[:, o:o + 512],
                                    start=True, stop=True)
                            dst = accs[p] if t == 0 else None
                            if dst is None:
                                if not dsts:
                                    tb_tmp = tmpp.tile([128, OWN], bf16, tag="tmp")
                                    dsts = [tb_tmp]
                                dst = dsts[0]
                            else:
                                dsts = [dst]
                            hv = hsv[:, 16 * j:16 * (j + 1), :]
                            dst3 = dst.rearrange("p (r v) -> p r v", v=WS)[
                                :, 16 * j:16 * (j + 1), :]
                            pw3 = pw_t.rearrange("p (r v) -> p r v", v=WS)
                            nc.vector.tensor_tensor(dst3, pw3, hv, OP.mult)
                        prod = dsts[0]
                    else:
                        wx = wxp.tile([128, OWN], bf16, tag="wx")
                        for j in range(2):
                            pw_t = pw_pool.tile([128, 1024], f32, tag="pw")
                            for jj in range(2):
                                o = j * 1024 + jj * 512
                                nc.tensor.matmul(
                                    pw_t[:, jj * 512:(jj + 1) * 512],
                                    w2ev[:, pt_i * COUT:(pt_i + 1) * COUT],
                                    r_sb[:, o:o + 512],
                                    start=True, stop=True)
                            nc.scalar.activation(wx[:, j * 1024:(j + 1) * 1024], pw_t,
                                                 AF.Copy)
                        wx3 = wx.rearrange("p (r v) -> p r v", v=WS)
                        if t == 0:
                            nc.vector.tensor_tensor(
                                accs[p].rearrange("p (r v) -> p r v", v=WS),
                                wx3, hsv, OP.mult)
                            prod = accs[p]
                        else:
                            prod = tmpp.tile([128, OWN], bf16, tag="tmp")
                            nc.vector.tensor_tensor(
                                prod.rearrange("p (r v) -> p r v", v=WS),
                                wx3, hsv, OP.mult)

                    if t == 0:
                        continue
                    nc.vector.tensor_tensor(acc, acc, prod, OP.add)

            # ---- DMA out (parity-major; host de-interleaves) ----
            for p in range(4):
                nc.sync.dma_start(
                    out=out_d[p],
                    in_=accs[p].rearrange("p (r v) -> p r v", v=WS))

    _split_multiwaits(nc, _nop_proto)
    return nc


def _split_multiwaits(nc, nop_proto):
    """The walrus build in this container rejects instructions carrying more
    than one sync-wait. Hoist extra waits onto injected same-engine NoOps
    (the sequencer executes them in order, so semantics are unchanged)."""
    import copy as _copy
    import bass_rust

    cnt = 0
    for f in nc.m.functions:
        for blk in f.blocks:
            new_list = []
            changed = False
            for inst in blk.instructions:
                si = inst.sync_info
                if si is not None and len(si.on_wait) > 1:
                    changed = True
                    waits = list(si.on_wait)
                    for w in waits[:-1]:
                        n = _copy.replace(nop_proto, name=f"WSPLIT-{cnt}")
                        cnt += 1
                        n.engine = inst.engine
                        n.sync_info = bass_rust.SyncInfo(on_wait=[w], on_update=[])
                        new_list.append(n)
                    inst.sync_info = bass_rust.SyncInfo(
                        on_wait=[waits[-1]], on_update=list(si.on_update))
                new_list.append(inst)
            if changed:
                blk.instructions = new_list


def _pack_bf16(arr):
    """[..., N] f32 -> [..., N/2] f32 whose bytes are the bf16 elements."""
    import ml_dtypes
    bf = np.asarray(arr, dtype=np.float32).astype(ml_dtypes.bfloat16)
    u16 = bf.view(np.uint16).astype(np.uint32)
    lo = u16[..., 0::2]
    hi = u16[..., 1::2]
    return (lo | (hi << 16)).view(np.float32)


def _host_prep(inputs):
    import ml_dtypes
    x = np.ascontiguousarray(np.asarray(inputs["x"], dtype=np.float32))
    w1x1 = np.asarray(inputs["w1x1"], dtype=np.float32)
    b1x1 = np.asarray(inputs["b1x1"], dtype=np.float32)
    w_red = np.asarray(inputs["w_red"], dtype=np.float32)
    b_red = np.asarray(inputs["b_red"], dtype=np.float32)
    bn_gamma = np.asarray(inputs["bn_gamma"], dtype=np.float32)
    bn_beta = np.asarray(inputs["bn_beta"], dtype=np.float32)
    bn_mean = np.asarray(inputs["bn_mean"], dtype=np.float32)
    bn_var = np.asarray(inputs["bn_var"], dtype=np.float32)
    w_span = np.asarray(inputs["w_span"], dtype=np.float32)
    b_span = np.asarray(inputs["b_span"], dtype=np.float32)

    a = bn_gamma / np.sqrt(bn_var + EPS)
    w_red_f = w_red * a[:, None]
    b_red_f = (b_red - bn_mean) * a + bn_beta

    w_span_g = w_span.reshape(G, K * K, CRED)
    b_span_g = b_span.reshape(G, K * K)
    w2 = np.zeros((2, 2, G, 3, 3, CRED), np.float32)
    b2 = np.zeros((2, 2, G, 3, 3), np.float32)
    for pa in range(2):
        for pb in range(2):
            for ky in range(K):
                dy = (pa + ky - 2) // 2
                for kx in range(K):
                    dx = (pb + kx - 2) // 2
                    w2[pa, pb, :, dy + 1, dx + 1] += w_span_g[:, ky * 5 + kx]
                    b2[pa, pb, :, dy + 1, dx + 1] += b_span_g[:, ky * 5 + kx]

    w2e = np.zeros((CRED + 1, 36 * COUT), np.float32)
    for p in range(4):
        pa, pb = p // 2, p % 2
        for t in range(9):
            ty, tx = t // 3, t % 3
            pt = p * 9 + t
            wexp = np.repeat(w2[pa, pb, :, ty, tx, :], GC, axis=0)  # [COUT, CRED]
            w2e[:CRED, pt * COUT:(pt + 1) * COUT] = wexp.T
            w2e[CRED, pt * COUT:(pt + 1) * COUT] = np.repeat(b2[pa, pb, :, ty, tx], GC)

    bf = ml_dtypes.bfloat16
    shared = np.zeros((128, FBLOB), np.float32)
    # conv1 weights, hi/lo split: [p, k*128+m] = w1x1.T[k*128+p, m]
    w1T = w1x1.T.reshape(2, 128, COUT).transpose(1, 0, 2).reshape(128, 256)
    w1T_hi = w1T.astype(bf).astype(np.float32)
    shared[:, O_W1HI:O_W1HI + 128] = _pack_bf16(w1T)
    shared[:, O_W1LO:O_W1LO + 128] = _pack_bf16(w1T - w1T_hi)
    shared[:, O_WR:O_WR + CRED] = w_red_f.T
    b1_hi = b1x1.astype(bf).astype(np.float32)
    shared[0, O_B1HI:O_B1HI + 64] = _pack_bf16(b1x1)
    shared[0, O_B1LO:O_B1LO + 64] = _pack_bf16(b1x1 - b1_hi)
    shared[0:CRED, O_BR] = b_red_f
    shared[0:CRED + 1, O_W2E:O_W2E + 36 * 64] = _pack_bf16(w2e)

    in_maps = []
    for core in range(NCORES):
        bi, hi = core // 2, core % 2
        u0 = hi * 32
        xs = np.zeros((CIN, ROWS, WS), np.float32)
        msk = np.zeros((ROWS, WS), np.float32)
        lo_r = max(0, u0 - 1)
        hi_r = min(HS, u0 + 33)
        d0 = lo_r - (u0 - 1)
        xs[:, d0:d0 + hi_r - lo_r] = x[bi, :, lo_r:hi_r]
        msk[d0:d0 + hi_r - lo_r] = 1.0
        blob = shared.copy()
        xsr = xs.reshape(2, 128, PX).transpose(1, 0, 2).reshape(128, 2 * PX)
        xs_hi = xsr.astype(bf).astype(np.float32)
        blob[:, O_XHI:O_XHI + PX] = _pack_bf16(xsr)
        blob[:, O_XLO:O_XLO + PX] = _pack_bf16(xsr - xs_hi)
        blob[0, O_MSK:O_MSK + PX // 2] = _pack_bf16(msk.reshape(PX))
        in_maps.append({"blob": blob})
    return in_maps


def kernel(**inputs):
    from concourse import bass_utils

    if "nc" not in _cache:
        _cache["nc"] = _build_module()
    nc = _cache["nc"]

    in_maps = _host_prep(inputs)
    res = bass_utils.run_bass_kernel_spmd(nc, in_maps, core_ids=list(range(NCORES)))

    out = np.zeros((B, COUT, 2 * HS, 2 * WS), np.float32)
    for core in range(NCORES):
        bi, hi = core // 2, core % 2
        r = np.asarray(res.results[core]["out"], dtype=np.float32)  # [4, C, 32, 64]
        r = r.reshape(2, 2, COUT, 32, WS).transpose(2, 3, 0, 4, 1)  # [C, u, pa, v, pb]
        out[bi, :, hi * 64:(hi + 1) * 64, :] = r.reshape(COUT, 64, 128)
    return out
